# revision 13
# baseline (speedup 1.0000x reference)
"""Trainium2 Bass kernel for BatchedDiffPoolLayer (gnn_message_passing).

Self-contained: hardcodes shapes/sharding. Inputs are the FULL arrays from
setup_inputs(); output is the full (adj_new, h) tuple.

Strategy (8 NeuronCores, dst-sharded nodes/edges):
  Phase 1: neigh segment-sum via dma_gather + onehot-matmul (dst-sorted edge
           groups of 512 with drifting base, slot width 64), f32.
  Phase 2: SAGE matmuls + double softmax -> r, rc, embed; h-partial matmul.
  Collective: AllGather compact rc [N,8]; expand to half-wide r_half (bf16).
  Phase 3: z = A^T r accumulation with same machinery (bf16 gathers).
  Phase 4: adj-partial = z^T-contract r; AllReduce (adj|h).
"""
import numpy as np
import ml_dtypes

N = 50000
E = 800000
FIN = 128
FOUT = 128
ASSIGN = 256
B = 32
NCORE = 8
SH = N // NCORE          # 6250
P = 128
SPLIT = 32767            # lo table rows 0..32766 + zero row at 32767
LO_ROWS = SPLIT + 1      # 32768
HI_REAL = N - SPLIT      # 17233 (nodes 32767..49999)
HI_ROWS = HI_REAL + 1    # 17234, zero row at 17233
HI_PAD = 20480           # 5*4096 (mask/rh_hi padded rows)
GROUP = 512
TPG = 4                  # 128-edge tiles per group
WIDTH = 64               # onehot slot width; group dst spread must be < WIDTH
CHUNK_E = 1024           # edges per dma_gather chunk (HW ring limit ~1024 idxs/call)
STREAM_CFGS = [(512, 64), (512, 128), (256, 128)]  # (GROUP, WIDTH) candidates
SHPAD = SH + 128         # neigh/z free width (bases + WIDTH <= SHPAD)
NT = (SH + P - 1) // P   # 49 node tiles, last has 106 valid rows
LASTV = SH - (NT - 1) * P  # 106
TCH = 7                  # node tiles per phase-2 chunk (49 = 7*7)
NCH = NT // TCH
F32 = np.float32
BF16 = ml_dtypes.bfloat16

_CACHE = {}


# ---------------------------------------------------------------- host prep
def _wrap_idx(stream):
    """Edge stream (len multiple of 16) -> dma_gather wrapped int16 [128, n/16]."""
    n = len(stream)
    w = stream.reshape(n // 16, 16).T.astype(np.int16)   # [16, n/16]
    return np.ascontiguousarray(np.tile(w, (8, 1)))      # [128, n/16]


def _build_stream(src_tbl_idx, dstloc, zero_idx, group, width):
    """Sort by dstloc, emit `group`-edge groups with dst spread < width."""
    order = np.argsort(dstloc, kind="stable")
    s = src_tbl_idx[order]
    d = dstloc[order]
    ne = len(s)
    out_s, out_r, bases = [], [], []
    i = 0
    while i < ne:
        base = int(d[i])
        j = min(i + group, ne)
        hi = int(np.searchsorted(d[i:j], base + width, side="left"))
        j = i + hi
        g_s = np.full(group, zero_idx, np.int64)
        g_r = np.zeros(group, np.int64)
        g_s[: j - i] = s[i:j]
        g_r[: j - i] = d[i:j] - base
        out_s.append(g_s)
        out_r.append(g_r)
        bases.append(base)
        i = j
    return out_s, out_r, bases


def _pad_streams(parts, zero_idx, G, group):
    out_s, out_r, bases = [list(x) for x in parts]
    assert len(bases) <= G
    while len(bases) < G:
        out_s.append(np.full(group, zero_idx, np.int64))
        out_r.append(np.zeros(group, np.int64))
        bases.append(0)
    stream = np.concatenate(out_s)
    rel = np.concatenate(out_r)
    T = G * (group // P)
    idx16 = _wrap_idx(stream)
    rel_pt = np.ascontiguousarray(rel.reshape(T, P).T.astype(F32))
    base_arr = np.array(bases, np.int32).reshape(1, G)
    return idx16, rel_pt, base_arr


def _pick_cfg(edge_sets):
    """edge_sets: list over cores of (src_idx, dstloc, zero). Pick (group,width)
    minimizing padded slots; returns (group, width, G)."""
    best = None
    for group, width in STREAM_CFGS:
        gmax = 1
        for s, d, z in edge_sets:
            _, _, bases = _build_stream(s, d, z, group, width)
            gmax = max(gmax, len(bases))
        cpg = CHUNK_E // group
        G = ((gmax + cpg - 1) // cpg) * cpg
        slots = G * group
        if best is None or slots < best[0]:
            best = (slots, group, width, G)
    return best[1], best[2], best[3]


def _host_prep(x, src, dst, batch, W_embed, b_embed, W_pool, b_pool):
    x = np.ascontiguousarray(x, F32)
    src = np.asarray(src, np.int64)
    dst = np.asarray(dst, np.int64)
    batch = np.asarray(batch, np.int64)

    x_lo = np.zeros((LO_ROWS, FIN), F32)
    x_lo[:SPLIT] = x[:SPLIT]
    x_hi = np.zeros((HI_ROWS, FIN), F32)
    x_hi[:HI_REAL] = x[SPLIT:]

    W_top = np.ascontiguousarray(np.concatenate([W_embed[:FIN], W_pool[:FIN]], axis=1), F32)
    W_bot = np.ascontiguousarray(np.concatenate([W_embed[FIN:], W_pool[FIN:]], axis=1), F32)
    bias = np.ascontiguousarray(np.concatenate([b_embed, b_pool]).reshape(1, 384), F32)

    iota64 = np.ascontiguousarray(
        np.tile(np.concatenate([np.tile(np.arange(64, dtype=F32), 4),
                                np.tile(np.arange(128, dtype=F32), 4)])[None, :], (P, 1)))
    # layout: cols 0:256 = iota64 x4, cols 256:768 = iota128 x4
    iota64_bf = iota64.astype(BF16)

    g16 = (batch % 16).astype(np.int64)
    blk = (np.arange(P) // 8)[None, :]
    m_lo = np.zeros((LO_ROWS, P), BF16)
    m_lo[:SPLIT] = (g16[:SPLIT, None] == blk).astype(BF16)
    m_hi = np.zeros((HI_PAD, P), BF16)
    m_hi[:HI_REAL] = (g16[SPLIT:, None] == blk).astype(BF16)

    half = (batch >= 16).astype(np.int64)

    per_core = []
    for c in range(NCORE):
        lo_n, hi_n = c * SH, (c + 1) * SH
        e = np.where((dst >= lo_n) & (dst < hi_n))[0]
        es, ed = src[e], dst[e] - lo_n
        deg = np.bincount(ed, minlength=SH).astype(F32)
        iv = np.ones(NT * P, F32)
        iv[:SH] = 1.0 / np.maximum(deg, 1.0)
        invdeg_pt = np.ascontiguousarray(iv.reshape(NT, P).T)
        mask = (batch[lo_n:hi_n, None] == (np.arange(ASSIGN) // 8)[None, :])
        maskf = np.zeros((NT * P, ASSIGN), F32)
        maskf[:SH] = mask.astype(F32)
        mask_pt = np.ascontiguousarray(
            maskf.reshape(NT, P, ASSIGN).transpose(1, 0, 2).reshape(P, NT * ASSIGN))
        xTfull = np.zeros((NT * P, FIN), F32)
        xTfull[:SH] = x[lo_n:hi_n]
        xT = np.ascontiguousarray(xTfull.T)               # [128, NT*P]

        elo = es < SPLIT
        eset = {}
        eset["p1_lo"] = (es[elo], ed[elo], SPLIT)
        eset["p1_hi"] = (es[~elo] - SPLIT, ed[~elo], HI_REAL)
        for h in range(2):
            sel = half[es] == h
            for t, tsel, zi, off in (("lo", elo, SPLIT, 0), ("hi", ~elo, HI_REAL, SPLIT)):
                m = sel & tsel
                eset[f"p3_{h}{t}"] = (es[m] - off, ed[m], zi)
        per_core.append(dict(eset=eset, invdeg=invdeg_pt, mask=mask_pt, xT=xT))

    KEYS = ["p1_lo", "p1_hi", "p3_0lo", "p3_0hi", "p3_1lo", "p3_1hi"]
    cfg = {}
    for k in KEYS:
        group, width, G = _pick_cfg([pc["eset"][k] for pc in per_core])
        cfg[k] = (group, width, G)

    in_maps = []
    for c, pc in enumerate(per_core):
        m = dict(x_lo=x_lo, x_hi=x_hi, w_top=W_top, w_bot=W_bot, bias_r=bias,
                 iota_f=iota64, iota_b=iota64_bf, m16lo=m_lo, m16hi=m_hi,
                 invdeg=pc["invdeg"], mask256=pc["mask"], xT=pc["xT"])
        for k in KEYS:
            group, width, G = cfg[k]
            s, d, zi = pc["eset"][k]
            parts = _build_stream(s, d, zi, group, width)
            i16, rel, ba = _pad_streams(parts, zi, G, group)
            m[f"{k}_idx"] = i16
            m[f"{k}_rel"] = rel.astype(BF16) if k.startswith("p3") else rel
            m[f"{k}_base"] = ba
        in_maps.append(m)
    return in_maps, cfg


# ---------------------------------------------------------------- device build
def _build_nc(cfg):
    import os
    PH = int(os.environ.get("K_PHASES", "4"))
    P1MODE = os.environ.get("K_P1MODE", "full")
    import concourse.bacc as bacc
    import concourse.bass as bass
    import concourse.mybir as mybir
    import concourse.tile as tile
    from concourse.masks import make_identity
    from ordered_set import OrderedSet

    f32 = mybir.dt.float32
    bf16 = mybir.dt.bfloat16
    i32 = mybir.dt.int32
    i16 = mybir.dt.int16
    AT = mybir.AluOpType
    AF = mybir.ActivationFunctionType
    DVE = OrderedSet([mybir.EngineType.DVE])

    nc = bacc.Bacc("TRN2", target_bir_lowering=False, debug=False,
                   enable_asserts=False, num_devices=NCORE)

    x_lo = nc.dram_tensor("x_lo", [LO_ROWS, FIN], f32, kind="ExternalInput")
    x_hi = nc.dram_tensor("x_hi", [HI_ROWS, FIN], f32, kind="ExternalInput")
    w_top = nc.dram_tensor("w_top", [FIN, 384], f32, kind="ExternalInput")
    w_bot = nc.dram_tensor("w_bot", [FIN, 384], f32, kind="ExternalInput")
    bias_r = nc.dram_tensor("bias_r", [1, 384], f32, kind="ExternalInput")
    iota_f = nc.dram_tensor("iota_f", [P, 768], f32, kind="ExternalInput")
    iota_b = nc.dram_tensor("iota_b", [P, 768], bf16, kind="ExternalInput")
    m16lo = nc.dram_tensor("m16lo", [LO_ROWS, P], bf16, kind="ExternalInput")
    m16hi = nc.dram_tensor("m16hi", [HI_PAD, P], bf16, kind="ExternalInput")
    invdeg = nc.dram_tensor("invdeg", [P, NT], f32, kind="ExternalInput")
    mask256 = nc.dram_tensor("mask256", [P, NT * ASSIGN], f32, kind="ExternalInput")
    xT_in = nc.dram_tensor("xT", [FIN, NT * P], f32, kind="ExternalInput")

    meta = {}
    for k, (group, width, G) in cfg.items():
        dt_rel = bf16 if k.startswith("p3") else f32
        tpg = group // P
        meta[f"{k}_idx"] = nc.dram_tensor(f"{k}_idx", [P, G * (group // 16)], i16, kind="ExternalInput")
        meta[f"{k}_rel"] = nc.dram_tensor(f"{k}_rel", [P, G * tpg], dt_rel, kind="ExternalInput")
        meta[f"{k}_base"] = nc.dram_tensor(f"{k}_base", [1, G], i32, kind="ExternalInput")

    out_adj = nc.dram_tensor("out_adj", [ASSIGN, ASSIGN], f32, kind="ExternalOutput")
    out_h = nc.dram_tensor("out_h", [ASSIGN, FOUT], f32, kind="ExternalOutput")
    dbg = nc.dram_tensor("dbg", [P, SHPAD], f32, kind="ExternalOutput")

    r_dram = nc.dram_tensor("r_dram", [NT * P, ASSIGN], f32)
    rc_cc = nc.dram_tensor("rc_cc", [SH, 8], bf16)
    rc_ag = nc.dram_tensor("rc_ag", [N, 8], bf16, addr_space="Shared")
    rh_lo = nc.dram_tensor("rh_lo", [LO_ROWS, P], bf16)
    rh_hi = nc.dram_tensor("rh_hi", [HI_PAD, P], bf16)
    cc2_in = nc.dram_tensor("cc2_in", [ASSIGN, 384], f32)
    cc2_out = nc.dram_tensor("cc2_out", [ASSIGN, 384], f32, addr_space="Shared")

    rg = [list(range(NCORE))]

    with tile.TileContext(nc) as tc:

        def scatter_phase(streams, acc_for, dtt, tbl_for, iota_t):
            with tc.tile_pool(name="scat", bufs=1) as sp, \
                 tc.tile_pool(name="gp", bufs=2) as gp, \
                 tc.tile_pool(name="ohp", bufs=4) as ohp, \
                 tc.tile_pool(name="psp", bufs=4, space="PSUM") as psp:
                for key in streams:
                    group, width, G = cfg[key]
                    tpg = group // P
                    cpg = CHUNK_E // group
                    io0 = 0 if width == 64 else 256
                    acc = acc_for(key)
                    tbl = tbl_for(key)
                    idx_sb = sp.tile([P, G * (group // 16)], i16, tag=f"idx{key}")
                    rel_sb = sp.tile([P, G * tpg], dtt, tag=f"rel{key}")
                    base_sb = sp.tile([1, G], i32, tag=f"base{key}")
                    nc.sync.dma_start(out=idx_sb[:], in_=meta[f"{key}_idx"][:])
                    nc.sync.dma_start(out=rel_sb[:], in_=meta[f"{key}_rel"][:])
                    nc.sync.dma_start(out=base_sb[:], in_=meta[f"{key}_base"][:])
                    for ch in range(G // cpg):
                        gt = gp.tile([P, CHUNK_E // P, FIN], dtt, tag="gt")
                        nc.gpsimd.dma_gather(
                            out_ap=gt[:],
                            in_ap=tbl[:],
                            idxs_ap=idx_sb[:, ch * (CHUNK_E // 16):(ch + 1) * (CHUNK_E // 16)],
                            num_idxs=CHUNK_E,
                            num_idxs_reg=CHUNK_E,
                            elem_size=FIN,
                        )
                        if P1MODE == "gather":
                            continue
                        for g in range(cpg):
                            gg = ch * cpg + g
                            oh = ohp.tile([P, 4 * 128], dtt, tag="oh")
                            relb = rel_sb[:, gg * tpg:(gg + 1) * tpg] \
                                .rearrange("p (g o) -> p g o", g=tpg, o=1) \
                                .to_broadcast([P, tpg, width])
                            nc.vector.tensor_tensor(
                                out=oh[:, 0:tpg * width].rearrange("p (g w) -> p g w", g=tpg, w=width),
                                in0=relb,
                                in1=iota_t[:, io0:io0 + tpg * width]
                                .rearrange("p (g w) -> p g w", g=tpg, w=width),
                                op=AT.is_equal)
                            if P1MODE == "onehot":
                                continue
                            ps = psp.tile([P, 128], f32, tag="ps")
                            for t in range(tpg):
                                nc.tensor.matmul(
                                    ps[:, 0:width],
                                    lhsT=gt[:, (g * tpg + t):(g * tpg + t + 1), :],
                                    rhs=oh[:, t * width:(t + 1) * width],
                                    start=(t == 0), stop=(t == tpg - 1))
                            if P1MODE == "matmul":
                                continue
                            if P1MODE == "add0":
                                bval = 0
                            else:
                                bval = nc.values_load(base_sb[0:1, gg:gg + 1], engines=DVE,
                                                      min_val=0, max_val=SHPAD - width,
                                                      skip_runtime_bounds_check=True)
                            nc.vector.tensor_tensor(
                                out=acc[:, bass.ds(bval, width)],
                                in0=acc[:, bass.ds(bval, width)],
                                in1=ps[:, 0:width], op=AT.add)

        with tc.tile_pool(name="always", bufs=1) as ap:
            wt = ap.tile([FIN, 384], f32, tag="wt")
            wb = ap.tile([FIN, 384], f32, tag="wb")
            br_t = ap.tile([1, 384], f32, tag="br")
            ones1 = ap.tile([1, P], f32, tag="ones1")
            iof = ap.tile([P, 768], f32, tag="iof")
            iob = ap.tile([P, 768], bf16, tag="iob")
            ivd = ap.tile([P, NT], f32, tag="ivd")
            ident = ap.tile([P, P], f32, tag="ident")
            nc.sync.dma_start(out=wt[:], in_=w_top[:])
            nc.sync.dma_start(out=wb[:], in_=w_bot[:])
            nc.sync.dma_start(out=br_t[:], in_=bias_r[:])
            nc.sync.dma_start(out=iof[:], in_=iota_f[:])
            nc.sync.dma_start(out=iob[:], in_=iota_b[:])
            nc.sync.dma_start(out=ivd[:], in_=invdeg[:])
            nc.vector.memset(ones1[:], 1.0)
            make_identity(nc, ident[:])

            with tc.tile_pool(name="hps", bufs=1, space="PSUM") as hps:
                ph0 = hps.tile([P, FOUT], f32, tag="ph0")
                ph1 = hps.tile([P, FOUT], f32, tag="ph1")

                # ================= phases 1 + 2 =================
                with tc.tile_pool(name="ph12", bufs=1) as pa:
                    neigh = pa.tile([P, SHPAD], f32, tag="neigh")
                    xT = pa.tile([FIN, NT * P], f32, tag="xT")
                    nc.vector.memset(neigh[:], 0.0)
                    nc.sync.dma_start(out=xT[:], in_=xT_in[:])

                    scatter_phase(["p1_lo", "p1_hi"],
                                  lambda k: neigh, f32,
                                  lambda k: x_lo if k.endswith("lo") else x_hi, iof)
                    nc.sync.dma_start(out=dbg[:], in_=neigh[:])

                    if PH >= 2:
                     with tc.tile_pool(name="p2", bufs=2) as p2, \
                         tc.tile_pool(name="p2ps", bufs=2, space="PSUM") as p2ps:
                        for chv in range(NCH):
                            pre = p2.tile([P, TCH * 384], f32, tag="pre")
                            mk = p2.tile([P, TCH * ASSIGN], f32, tag="mk")
                            e1 = p2.tile([P, TCH * ASSIGN], f32, tag="e1")
                            rb = p2.tile([P, TCH * ASSIGN], f32, tag="rb")
                            rcb = p2.tile([P, TCH * 8], f32, tag="rcb")
                            rcb16c = p2.tile([P, TCH * 8], bf16, tag="rcb16c")
                            s1 = p2.tile([P, TCH], f32, tag="s1")
                            iv1 = p2.tile([P, TCH], f32, tag="iv1")
                            s2 = p2.tile([P, TCH], f32, tag="s2")
                            rr2 = p2.tile([P, TCH], f32, tag="rr2")
                            nc.sync.dma_start(
                                out=mk[:],
                                in_=mask256[:, chv * TCH * ASSIGN:(chv + 1) * TCH * ASSIGN])
                            for tl in range(TCH):
                                tg = chv * TCH + tl
                                px = p2ps.tile([P, 384], f32, tag="px")
                                pn = p2ps.tile([P, 384], f32, tag="pn")
                                nc.tensor.matmul(px[:], lhsT=xT[:, tg * P:(tg + 1) * P],
                                                 rhs=wt[:], start=True, stop=False)
                                nc.tensor.matmul(px[:], lhsT=ones1[0:1, :], rhs=br_t[0:1, :],
                                                 start=False, stop=True)
                                nc.tensor.matmul(pn[:], lhsT=neigh[:, tg * P:(tg + 1) * P],
                                                 rhs=wb[:], start=True, stop=True)
                                nc.vector.tensor_scalar(
                                    out=pre[:, tl * 384:(tl + 1) * 384],
                                    in0=pn[:], scalar1=ivd[:, tg:tg + 1], scalar2=None,
                                    op0=AT.mult)
                                nc.vector.tensor_tensor(
                                    out=pre[:, tl * 384:(tl + 1) * 384],
                                    in0=pre[:, tl * 384:(tl + 1) * 384],
                                    in1=px[:], op=AT.add)
                                nc.scalar.activation(
                                    e1[:, tl * ASSIGN:(tl + 1) * ASSIGN],
                                    pre[:, tl * 384 + FOUT:(tl + 1) * 384],
                                    AF.Exp, accum_out=s1[:, tl:tl + 1])
                            nc.vector.reciprocal(iv1[:], s1[:])
                            for tl in range(TCH):
                                nc.scalar.activation(
                                    rb[:, tl * ASSIGN:(tl + 1) * ASSIGN],
                                    e1[:, tl * ASSIGN:(tl + 1) * ASSIGN],
                                    AF.Exp, scale=iv1[:, tl:tl + 1])
                                nc.vector.scalar_tensor_tensor(
                                    out=rb[:, tl * ASSIGN:(tl + 1) * ASSIGN],
                                    in0=rb[:, tl * ASSIGN:(tl + 1) * ASSIGN],
                                    scalar=1.0, in1=mk[:, tl * ASSIGN:(tl + 1) * ASSIGN],
                                    op0=AT.mult, op1=AT.mult,
                                    accum_out=s2[:, tl:tl + 1])
                            nc.vector.tensor_scalar(out=s2[:], in0=s2[:], scalar1=1e-13,
                                                    scalar2=None, op0=AT.add)
                            nc.vector.reciprocal(rr2[:], s2[:])
                            for tl in range(TCH):
                                nc.scalar.activation(
                                    rb[:, tl * ASSIGN:(tl + 1) * ASSIGN],
                                    rb[:, tl * ASSIGN:(tl + 1) * ASSIGN],
                                    AF.Copy, scale=rr2[:, tl:tl + 1])
                            nc.vector.tensor_reduce(
                                rcb[:].rearrange("p (t j) -> p t j", t=TCH, j=8),
                                rb[:].rearrange("p (t b j) -> p t j b", t=TCH, b=B, j=8),
                                mybir.AxisListType.X, AT.add)
                            nc.vector.tensor_copy(out=rcb16c[:], in_=rcb[:])
                            for tl in range(TCH):
                                tg = chv * TCH + tl
                                nc.tensor.matmul(
                                    ph0[:], lhsT=rb[:, tl * ASSIGN:tl * ASSIGN + P],
                                    rhs=pre[:, tl * 384:tl * 384 + FOUT],
                                    start=(tg == 0), stop=(tg == NT - 1),
                                    skip_group_check=True)
                                nc.tensor.matmul(
                                    ph1[:], lhsT=rb[:, tl * ASSIGN + P:(tl + 1) * ASSIGN],
                                    rhs=pre[:, tl * 384:tl * 384 + FOUT],
                                    start=(tg == 0), stop=(tg == NT - 1),
                                    skip_group_check=True)
                                nc.sync.dma_start(out=r_dram[tg * P:(tg + 1) * P, :],
                                                  in_=rb[:, tl * ASSIGN:(tl + 1) * ASSIGN])
                                rows = LASTV if tg == NT - 1 else P
                                nc.sync.dma_start(out=rc_cc[tg * P:tg * P + rows, :],
                                                  in_=rcb16c[0:rows, tl * 8:(tl + 1) * 8])

                # ================= collective + expansion =================
                if PH >= 3:
                    nc.gpsimd.collective_compute(
                        "AllGather", AT.bypass, replica_groups=rg,
                        ins=[rc_cc[:]], outs=[rc_ag[:]])

                if PH >= 3 and os.environ.get("K_EXP", "1") == "1":
                 with tc.tile_pool(name="exp", bufs=3) as ep:
                    EIT = 4096
                    ET = EIT // P                              # 32 tiles / iter
                    def expand_iter(dst, msk, node0, nreal, orow):
                        rct = ep.tile([P, ET, 8], bf16, tag="rct")
                        if nreal < EIT:
                            nc.vector.memset(rct[:], 0.0)
                            full_t = nreal // P
                            if full_t:
                                nc.sync.dma_start(
                                    out=rct[:, 0:full_t, :],
                                    in_=rc_ag[node0:node0 + full_t * P, :]
                                    .rearrange("(t p) j -> p t j", p=P, t=full_t, j=8))
                            remv = nreal - full_t * P
                            if remv:
                                nc.sync.dma_start(
                                    out=rct[0:remv, full_t:full_t + 1, :],
                                    in_=rc_ag[node0 + full_t * P:node0 + nreal, :]
                                    .rearrange("(t p) j -> p t j", p=remv, t=1, j=8))
                        else:
                            nc.sync.dma_start(
                                out=rct[:],
                                in_=rc_ag[node0:node0 + EIT, :]
                                .rearrange("(t p) j -> p t j", p=P, t=ET, j=8))
                        mt = ep.tile([P, ET * P], bf16, tag="mt")
                        nc.scalar.dma_start(
                            out=mt[:].rearrange("p (t w) -> p t w", t=ET, w=P),
                            in_=msk[orow:orow + EIT, :]
                            .rearrange("(t p) w -> p t w", p=P, t=ET, w=P))
                        rh = ep.tile([P, ET * P], bf16, tag="rh")
                        nc.vector.tensor_tensor(
                            out=rh[:].rearrange("p (t w) -> p t w", t=ET, w=P),
                            in0=rct[:].rearrange("p t (o j) -> p t o j", o=1, j=8)
                            .to_broadcast([P, ET, 16, 8]),
                            in1=mt[:].rearrange("p (t g j) -> p t g j", t=ET, g=16, j=8),
                            op=AT.mult)
                        nc.gpsimd.dma_start(
                            out=dst[orow:orow + EIT, :]
                            .rearrange("(t p) w -> p t w", p=P, t=ET, w=P),
                            in_=rh[:].rearrange("p (t w) -> p t w", t=ET, w=P))

                    for it in range(8):                       # lo rows 0..32767
                        r0 = it * EIT
                        expand_iter(rh_lo, m16lo, r0, min(EIT, SPLIT - r0), r0)
                    for it in range(5):                       # hi: nodes 32767..49999
                        n0 = SPLIT + it * EIT
                        expand_iter(rh_hi, m16hi, n0, min(EIT, N - n0), n0 - SPLIT)

                # ================= phases 3 + 4 =================
                if PH < 4:
                    with tc.tile_pool(name="fb", bufs=1) as fb:
                        za = fb.tile([P, 2 * ASSIGN], f32, tag="za")
                        nc.vector.memset(za[:], 0.0)
                        nc.sync.dma_start(
                            out=out_adj[:].rearrange("(c p) f -> p c f", c=2, p=P, f=ASSIGN),
                            in_=za[:].rearrange("p (c f) -> p c f", c=2, f=ASSIGN))
                        nc.sync.dma_start(
                            out=out_h[:].rearrange("(c p) f -> p c f", c=2, p=P, f=FOUT),
                            in_=za[:, 0:2 * FOUT].rearrange("p (c f) -> p c f", c=2, f=FOUT))
                if PH >= 4:
                 with tc.tile_pool(name="ph34", bufs=1) as pz:
                    z_lo = pz.tile([P, SHPAD], f32, tag="zlo")
                    z_hi = pz.tile([P, SHPAD], f32, tag="zhi")
                    nc.vector.memset(z_lo[:], 0.0)
                    nc.vector.memset(z_hi[:], 0.0)

                    scatter_phase(["p3_0lo", "p3_0hi", "p3_1lo", "p3_1hi"],
                                  lambda k: z_lo if k[3] == "0" else z_hi, bf16,
                                  lambda k: rh_lo if k.endswith("lo") else rh_hi, iob)

                    with tc.tile_pool(name="p4", bufs=3) as p4, \
                         tc.tile_pool(name="p4ps", bufs=4, space="PSUM") as p4ps, \
                         tc.tile_pool(name="adjps", bufs=1, space="PSUM") as adjps:
                        pa0 = adjps.tile([P, ASSIGN], f32, tag="pa0")
                        pa1 = adjps.tile([P, ASSIGN], f32, tag="pa1")
                        for tg in range(NT):
                            rt = p4.tile([P, ASSIGN], f32, tag="rt")
                            nc.sync.dma_start(out=rt[:], in_=r_dram[tg * P:(tg + 1) * P, :])
                            for zsrc, padj in ((z_lo, pa0), (z_hi, pa1)):
                                pt = p4ps.tile([P, P], f32, tag="pt")
                                nc.tensor.transpose(out=pt[:],
                                                    in_=zsrc[:, tg * P:(tg + 1) * P],
                                                    identity=ident[:])
                                zt = p4.tile([P, P], f32, tag="zt")
                                nc.vector.tensor_copy(out=zt[:], in_=pt[:])
                                nc.tensor.matmul(padj[:], lhsT=zt[:], rhs=rt[:],
                                                 start=(tg == 0), stop=(tg == NT - 1),
                                                 skip_group_check=True)
                        comb = p4.tile([P, 768], f32, tag="comb")
                        nc.vector.tensor_copy(out=comb[:, 0:ASSIGN], in_=pa0[:])
                        nc.vector.tensor_copy(out=comb[:, ASSIGN:384], in_=ph0[:])
                        nc.vector.tensor_copy(out=comb[:, 384:384 + ASSIGN], in_=pa1[:])
                        nc.vector.tensor_copy(out=comb[:, 384 + ASSIGN:768], in_=ph1[:])
                        nc.sync.dma_start(
                            out=cc2_in[:].rearrange("(c p) f -> p c f", c=2, p=P, f=384),
                            in_=comb[:].rearrange("p (c f) -> p c f", c=2, f=384))
                        nc.gpsimd.collective_compute(
                            "AllReduce", AT.add, replica_groups=rg,
                            ins=[cc2_in[:]], outs=[cc2_out[:]])
                        fin = p4.tile([P, 768], f32, tag="fin")
                        nc.sync.dma_start(
                            out=fin[:].rearrange("p (c f) -> p c f", c=2, f=384),
                            in_=cc2_out[:].rearrange("(c p) f -> p c f", c=2, p=P, f=384))
                        nc.sync.dma_start(
                            out=out_adj[:].rearrange("(c p) f -> p c f", c=2, p=P, f=ASSIGN),
                            in_=fin[:].rearrange("p (c f) -> p c f", c=2, f=384)[:, :, 0:ASSIGN])
                        nc.sync.dma_start(
                            out=out_h[:].rearrange("(c p) f -> p c f", c=2, p=P, f=FOUT),
                            in_=fin[:].rearrange("p (c f) -> p c f", c=2, f=384)[:, :, ASSIGN:384])

    nc.compile()
    return nc


# ---------------------------------------------------------------- entry point
def kernel(x, edge_index, batch, W_embed, b_embed, W_pool, b_pool):
    from concourse.bass_utils import run_bass_kernel_spmd

    x = np.asarray(x, F32)
    ei = np.asarray(edge_index).astype(np.int64)
    batch = np.asarray(batch).astype(np.int64)
    W_embed = np.asarray(W_embed, F32)
    b_embed = np.asarray(b_embed, F32)
    W_pool = np.asarray(W_pool, F32)
    b_pool = np.asarray(b_pool, F32)

    in_maps, cfg = _host_prep(x, ei[0], ei[1], batch,
                              W_embed, b_embed, W_pool, b_pool)
    key = tuple(sorted(cfg.items()))
    if key not in _CACHE:
        _CACHE[key] = _build_nc(cfg)
    nc = _CACHE[key]
    res = run_bass_kernel_spmd(nc, in_maps, core_ids=list(range(NCORE)))
    adj = np.asarray(res.results[0]["out_adj"], F32)
    h = np.asarray(res.results[0]["out_h"], F32)
    return adj, h
